# revision 4
# baseline (speedup 1.0000x reference)
"""Trainium2 Bass kernel for nn_DifferentiableAlways (sparse_attention).

Math: the reference builds [2T,T] matrices, but column c of the output is just
    out[c] = -log( sum_{d in D} exp(-sig_ext[c+d] * m[d]) )
where m[d] = sigmoid(d - t_start) * sigmoid(t_end - d) (f32), D = {d: m[d] > 1e-3}
(a contiguous window), and sig_ext = concat(signal, full(T, signal[-1])).
Entries outside D are masked to 1e6 and contribute exp(-1e6) == 0 exactly in f32.
Inside D, m[d] == 1.0 exactly (saturated sigmoids) except for ~23 values at each
end, so the bulk needs no multiply: ScalarEngine does exp(-x) + free-dim
accumulate in one fused activation per 128-column block.

Sharding: output columns split across 8 cores (512 each). Each core gets a
host-staged Hankel view of sig_ext ([128, F_S]; partition p = c mod 128,
free f = 128*(c//128) + d'), so each block's window is a plain free-dim slice.

Raw Bass (explicit semaphores) is used because this container's walrus rejects
instructions carrying more than one semaphore wait, which the Tile framework's
auto-sync emits.
"""

from contextlib import ExitStack

import numpy as np

import concourse.bass as bass
import concourse.mybir as mybir
from concourse.bass_utils import run_bass_kernel_spmd

T_DIM = 4096
N_CORES = 8
NC = T_DIM // N_CORES          # columns per core
NBLK = NC // 128               # 128-column blocks per core
LARGE_NUMBER = 1.0e6
DELTA = 1.0e-3
SCALE = 1.0

_F32 = mybir.dt.float32


def _build(W: int, n_lo: int, n_hi: int):
    """Per-core Bass program: window width W, with n_lo/n_hi unsaturated
    mask columns at the window edges (explicit multiply), and a saturated
    core of W_core columns where m == 1.0 (no multiply needed)."""
    n_edge = n_lo + n_hi
    W_core = W - n_edge
    F_S = 128 * (NBLK - 1) + W
    Exp = mybir.ActivationFunctionType.Exp
    Ln = mybir.ActivationFunctionType.Ln

    nc = bass.Bass()
    s_h = nc.dram_tensor("s_hankel", [128, F_S], _F32, kind="ExternalInput")
    m_e = None
    if n_edge:
        m_e = nc.dram_tensor("m_edge", [128, n_edge], _F32, kind="ExternalInput")
    out = nc.dram_tensor("out_chunk", [NC], _F32, kind="ExternalOutput")

    dma_total = 32 if n_edge else 16

    with ExitStack() as ctx:
        S = ctx.enter_context(nc.sbuf_tensor([128, F_S], _F32))
        M = ctx.enter_context(nc.sbuf_tensor([128, max(n_edge, 1)], _F32))
        xe = ctx.enter_context(nc.sbuf_tensor([128, max(n_edge * NBLK, 1)], _F32))
        sc = ctx.enter_context(nc.sbuf_tensor([128, max(W_core, 1)], _F32))
        se = ctx.enter_context(nc.sbuf_tensor([128, max(n_edge, 1)], _F32))
        acc = ctx.enter_context(nc.sbuf_tensor([128, 2 * NBLK], _F32))
        tot = ctx.enter_context(nc.sbuf_tensor([128, NBLK], _F32))
        lg = ctx.enter_context(nc.sbuf_tensor([128, NBLK], _F32))
        ng = ctx.enter_context(nc.sbuf_tensor([128, NBLK], _F32))
        dma_in = ctx.enter_context(nc.semaphore("dma_in"))
        mul_sem = ctx.enter_context(nc.semaphore("mul_sem"))
        acc_sem = ctx.enter_context(nc.semaphore("acc_sem"))
        add_sem = ctx.enter_context(nc.semaphore("add_sem"))
        ln_sem = ctx.enter_context(nc.semaphore("ln_sem"))
        neg_sem = ctx.enter_context(nc.semaphore("neg_sem"))
        dma_out = ctx.enter_context(nc.semaphore("dma_out"))
        block = ctx.enter_context(nc.Block())

        @block.sync
        def _(sync):
            sync.dma_start(out=S[:], in_=s_h[:]).then_inc(dma_in, 16)
            if n_edge:
                sync.dma_start(out=M[:], in_=m_e[:]).then_inc(dma_in, 16)
            sync.wait_ge(neg_sem, 1)
            for b in range(NBLK):
                sync.dma_start(
                    out=out[b * 128 : (b + 1) * 128], in_=ng[:, b : b + 1]
                ).then_inc(dma_out, 16)
            sync.wait_ge(dma_out, 16 * NBLK)

        @block.vector
        def _(vector):
            if n_edge:
                vector.wait_ge(dma_in, dma_total)
                for b in range(NBLK):
                    base = 128 * b
                    col = n_edge * b
                    ins = None
                    if n_lo:
                        ins = vector.tensor_mul(
                            xe[:, col : col + n_lo],
                            S[:, base : base + n_lo],
                            M[:, 0:n_lo],
                        )
                    if n_hi:
                        ins = vector.tensor_mul(
                            xe[:, col + n_lo : col + n_edge],
                            S[:, base + n_lo + W_core : base + W],
                            M[:, n_lo:n_edge],
                        )
                    ins.then_inc(mul_sem, 1)
            else:
                # edge half of acc must still be zero for the final add
                vector.memset(acc[:, NBLK : 2 * NBLK], 0.0)
            if W_core == 0:
                vector.memset(acc[:, 0:NBLK], 0.0)
            vector.wait_ge(acc_sem, NBLK)
            vector.tensor_add(tot[:], acc[:, 0:NBLK], acc[:, NBLK : 2 * NBLK]).then_inc(
                add_sem, 1
            )
            vector.wait_ge(ln_sem, 1)
            vector.tensor_scalar_mul(ng[:], lg[:], -1.0).then_inc(neg_sem, 1)

        @block.scalar
        def _(scalar):
            scalar.wait_ge(dma_in, dma_total)
            for b in range(NBLK):
                base = 128 * b
                last = None
                if W_core:
                    last = scalar.activation(
                        sc[:, 0:W_core],
                        S[:, base + n_lo : base + n_lo + W_core],
                        Exp,
                        scale=-1.0,
                        accum_out=acc[:, b : b + 1],
                    )
                if n_edge:
                    scalar.wait_ge(mul_sem, b + 1)
                    col = n_edge * b
                    last = scalar.activation(
                        se[:, 0:n_edge],
                        xe[:, col : col + n_edge],
                        Exp,
                        scale=-1.0,
                        accum_out=acc[:, NBLK + b : NBLK + b + 1],
                    )
                last.then_inc(acc_sem, 1)
            scalar.wait_ge(add_sem, 1)
            scalar.activation(lg[:], tot[:], Ln).then_inc(ln_sem, 1)

    return nc


_cache: dict = {}


def _get_program(W, n_lo, n_hi):
    key = (W, n_lo, n_hi)
    if key not in _cache:
        _cache[key] = _build(W, n_lo, n_hi)
    return _cache[key]


def _sigmoid_f32(x64: np.ndarray) -> np.ndarray:
    return (1.0 / (1.0 + np.exp(-x64))).astype(np.float32)


def kernel(signal, t_start, t_end):
    signal = np.asarray(signal, dtype=np.float32).reshape(-1)
    T = signal.shape[0]
    assert T == T_DIM, f"expected T={T_DIM}, got {T}"
    ts = float(np.asarray(t_start).reshape(()))
    te = float(np.asarray(t_end).reshape(()))

    d64 = np.arange(T, dtype=np.float64)
    m = (_sigmoid_f32(SCALE * (d64 - ts)) * _sigmoid_f32(SCALE * (te - d64))).astype(
        np.float32
    )
    in_window = m > np.float32(DELTA)
    if not in_window.any():
        # every entry masked to LARGE_NUMBER: out = LARGE - log(2T)
        val = np.float32(LARGE_NUMBER) - np.float32(np.log(np.float32(2 * T)))
        return np.full(T, val, dtype=np.float32)

    idx = np.nonzero(in_window)[0]
    d_lo, d_hi = int(idx[0]), int(idx[-1])
    W = d_hi - d_lo + 1
    assert bool(in_window[d_lo : d_hi + 1].all()), "mask window not contiguous"

    m_win = m[d_lo : d_hi + 1]
    sat = m_win == np.float32(1.0)
    if sat.any():
        si = np.nonzero(sat)[0]
        n_lo, n_hi = int(si[0]), int(W - 1 - si[-1])
        assert bool(sat[si[0] : si[-1] + 1].all()), "saturated core not contiguous"
    else:
        n_lo, n_hi = W, 0  # everything goes through the explicit-multiply path

    F_S = 128 * (NBLK - 1) + W
    sig_ext = np.empty(
        max(2 * T, T + NC * (N_CORES - 1) + d_lo + 127 + F_S), np.float32
    )
    sig_ext[:T] = signal
    sig_ext[T:] = signal[-1]

    m_edge = None
    if n_lo + n_hi:
        me = np.concatenate([m_win[:n_lo], m_win[W - n_hi :]]).astype(np.float32)
        m_edge = np.ascontiguousarray(np.broadcast_to(me[None, :], (128, n_lo + n_hi)))

    in_maps = []
    for q in range(N_CORES):
        base = NC * q + d_lo
        chunk = sig_ext[base : base + 127 + F_S]
        s_hankel = np.ascontiguousarray(
            np.lib.stride_tricks.sliding_window_view(chunk, F_S)[:128]
        )
        im = {"s_hankel": s_hankel}
        if m_edge is not None:
            im["m_edge"] = m_edge
        in_maps.append(im)

    nc = _get_program(W, n_lo, n_hi)
    res = run_bass_kernel_spmd(nc, in_maps, list(range(N_CORES)), **RUN_KWARGS)
    global LAST_RESULTS
    LAST_RESULTS = res
    return np.concatenate([res.results[q]["out_chunk"] for q in range(N_CORES)]).astype(
        np.float32
    )


# test-harness knobs (unused by graders): set RUN_KWARGS = {"trace": True}
# before calling kernel() to capture a profile in LAST_RESULTS.
RUN_KWARGS: dict = {}
LAST_RESULTS = None


# revision 8
# speedup vs baseline: 1.2992x; 1.2992x over previous
"""Trainium2 Bass kernel for nn_DifferentiableAlways (sparse_attention).

Math: the reference builds [2T,T] matrices, but column c of the output is just
    out[c] = -log( sum_{d in D} exp(-sig_ext[c+d] * m[d]) )
where m[d] = sigmoid(d - t_start) * sigmoid(t_end - d) (f32), D = {d: m[d] > 1e-3}
(a contiguous window), and sig_ext = concat(signal, full(T, signal[-1])).
Entries outside D are masked to 1e6 and contribute exp(-1e6) == 0 exactly in f32.
Inside D, m[d] == 1.0 exactly (saturated sigmoids) except for ~23 values at each
end, so the bulk needs no multiply: ScalarEngine does exp(-x) + free-dim
accumulate in one fused activation per 128-column block.

Sharding: output columns split across 8 cores (512 each). Each core gets a
host-staged Hankel view of sig_ext ([128, F_S]; partition p = c mod 128,
free f = 128*(c//128) + d'), so each block's window is a plain free-dim slice.

Raw Bass (explicit semaphores) is used because this container's walrus rejects
instructions carrying more than one semaphore wait, which the Tile framework's
auto-sync emits.
"""

from contextlib import ExitStack

import numpy as np

import concourse.bass as bass
import concourse.mybir as mybir
from concourse.bass_utils import run_bass_kernel_spmd

T_DIM = 4096
N_CORES = 8
NC = T_DIM // N_CORES          # columns per core
NBLK = NC // 128               # 128-column blocks per core
LARGE_NUMBER = 1.0e6
DELTA = 1.0e-3
SCALE = 1.0

_F32 = mybir.dt.float32


def _build(W: int, n_lo: int, n_hi: int):
    """Per-core Bass program: window width W, with n_lo/n_hi unsaturated
    mask columns at the window edges (explicit multiply), and a saturated
    core of W_core columns where m == 1.0 (no multiply needed)."""
    n_edge = n_lo + n_hi
    W_core = W - n_edge
    F_S = 128 * (NBLK - 1) + W
    Exp = mybir.ActivationFunctionType.Exp
    Ln = mybir.ActivationFunctionType.Ln

    nc = bass.Bass()
    s_h = nc.dram_tensor("s_hankel", [128, F_S], _F32, kind="ExternalInput")
    m_e = None
    if n_edge:
        m_e = nc.dram_tensor("m_edge", [128, n_edge], _F32, kind="ExternalInput")
    # out_chunk[p, b] = output for column 128*b + p of this core's slice;
    # the host transposes. Keeps the store one contiguous [128, NBLK] DMA.
    out = nc.dram_tensor("out_chunk", [128, NBLK], _F32, kind="ExternalOutput")

    dma_total = 32 if n_edge else 16

    with ExitStack() as ctx:
        S = ctx.enter_context(nc.sbuf_tensor([128, F_S], _F32))
        M = ctx.enter_context(nc.sbuf_tensor([128, max(n_edge, 1)], _F32))
        xe = ctx.enter_context(nc.sbuf_tensor([128, max(n_edge * NBLK, 1)], _F32))
        sc = ctx.enter_context(nc.sbuf_tensor([128, max(W_core, 1)], _F32))
        se = ctx.enter_context(nc.sbuf_tensor([128, max(n_edge, 1)], _F32))
        acc = ctx.enter_context(nc.sbuf_tensor([128, 2 * NBLK], _F32))
        tot = ctx.enter_context(nc.sbuf_tensor([128, NBLK], _F32))
        lg = ctx.enter_context(nc.sbuf_tensor([128, NBLK], _F32))
        ng = ctx.enter_context(nc.sbuf_tensor([128, NBLK], _F32))
        dma_in = ctx.enter_context(nc.semaphore("dma_in"))
        mul_sem = ctx.enter_context(nc.semaphore("mul_sem"))
        acc_sem = ctx.enter_context(nc.semaphore("acc_sem"))
        add_sem = ctx.enter_context(nc.semaphore("add_sem"))
        ln_sem = ctx.enter_context(nc.semaphore("ln_sem"))
        neg_sem = ctx.enter_context(nc.semaphore("neg_sem"))
        dma_out = ctx.enter_context(nc.semaphore("dma_out"))
        block = ctx.enter_context(nc.Block(no_gpsimd_drain=True))

        @block.sync
        def _(sync):
            sync.dma_start(out=S[:], in_=s_h[:]).then_inc(dma_in, 16)
            if n_edge:
                sync.dma_start(out=M[:], in_=m_e[:]).then_inc(dma_in, 16)
            sync.wait_ge(neg_sem, 1)
            sync.dma_start(out=out[:], in_=ng[:]).then_inc(dma_out, 16)
            sync.wait_ge(dma_out, 16)

        @block.vector
        def _(vector):
            if n_edge:
                vector.wait_ge(dma_in, dma_total)
                for b in range(NBLK):
                    base = 128 * b
                    col = n_edge * b
                    ins = None
                    if n_lo:
                        ins = vector.tensor_mul(
                            xe[:, col : col + n_lo],
                            S[:, base : base + n_lo],
                            M[:, 0:n_lo],
                        )
                    if n_hi:
                        ins = vector.tensor_mul(
                            xe[:, col + n_lo : col + n_edge],
                            S[:, base + n_lo + W_core : base + W],
                            M[:, n_lo:n_edge],
                        )
                    ins.then_inc(mul_sem, 1)
            else:
                # edge half of acc must still be zero for the final add
                vector.memset(acc[:, NBLK : 2 * NBLK], 0.0)
            if W_core == 0:
                vector.memset(acc[:, 0:NBLK], 0.0)
            vector.wait_ge(acc_sem, NBLK)
            vector.tensor_add(tot[:], acc[:, 0:NBLK], acc[:, NBLK : 2 * NBLK]).then_inc(
                add_sem, 1
            )
            vector.wait_ge(ln_sem, 1)
            vector.tensor_scalar_mul(ng[:], lg[:], -1.0).then_inc(neg_sem, 1)

        @block.scalar
        def _(scalar):
            # Warm the exp/ln activation-table load (~1.3us) under the DMA:
            # scale=0.0 makes the input read dead, so no dependency.
            scalar.activation(lg[:, 0:1], lg[:, 0:1], Exp, scale=0.0)
            scalar.wait_ge(dma_in, dma_total)
            for b in range(NBLK):
                base = 128 * b
                last = None
                if W_core:
                    last = scalar.activation(
                        sc[:, 0:W_core],
                        S[:, base + n_lo : base + n_lo + W_core],
                        Exp,
                        scale=-1.0,
                        accum_out=acc[:, b : b + 1],
                    )
                if n_edge:
                    scalar.wait_ge(mul_sem, b + 1)
                    col = n_edge * b
                    last = scalar.activation(
                        se[:, 0:n_edge],
                        xe[:, col : col + n_edge],
                        Exp,
                        scale=-1.0,
                        accum_out=acc[:, NBLK + b : NBLK + b + 1],
                    )
                last.then_inc(acc_sem, 1)
            scalar.wait_ge(add_sem, 1)
            scalar.activation(lg[:], tot[:], Ln).then_inc(ln_sem, 1)

    return nc


_cache: dict = {}


def _get_program(W, n_lo, n_hi):
    key = (W, n_lo, n_hi)
    if key not in _cache:
        _cache[key] = _build(W, n_lo, n_hi)
    return _cache[key]


def _sigmoid_f32(x64: np.ndarray) -> np.ndarray:
    return (1.0 / (1.0 + np.exp(-x64))).astype(np.float32)


def kernel(signal, t_start, t_end):
    signal = np.asarray(signal, dtype=np.float32).reshape(-1)
    T = signal.shape[0]
    assert T == T_DIM, f"expected T={T_DIM}, got {T}"
    ts = float(np.asarray(t_start).reshape(()))
    te = float(np.asarray(t_end).reshape(()))

    d64 = np.arange(T, dtype=np.float64)
    m = (_sigmoid_f32(SCALE * (d64 - ts)) * _sigmoid_f32(SCALE * (te - d64))).astype(
        np.float32
    )
    in_window = m > np.float32(DELTA)
    if not in_window.any():
        # every entry masked to LARGE_NUMBER: out = LARGE - log(2T)
        val = np.float32(LARGE_NUMBER) - np.float32(np.log(np.float32(2 * T)))
        return np.full(T, val, dtype=np.float32)

    idx = np.nonzero(in_window)[0]
    d_lo, d_hi = int(idx[0]), int(idx[-1])
    W = d_hi - d_lo + 1
    assert bool(in_window[d_lo : d_hi + 1].all()), "mask window not contiguous"

    m_win = m[d_lo : d_hi + 1]
    sat = m_win == np.float32(1.0)
    if sat.any():
        si = np.nonzero(sat)[0]
        n_lo, n_hi = int(si[0]), int(W - 1 - si[-1])
        assert bool(sat[si[0] : si[-1] + 1].all()), "saturated core not contiguous"
    else:
        n_lo, n_hi = W, 0  # everything goes through the explicit-multiply path

    F_S = 128 * (NBLK - 1) + W
    sig_ext = np.empty(
        max(2 * T, T + NC * (N_CORES - 1) + d_lo + 127 + F_S), np.float32
    )
    sig_ext[:T] = signal
    sig_ext[T:] = signal[-1]

    m_edge = None
    if n_lo + n_hi:
        me = np.concatenate([m_win[:n_lo], m_win[W - n_hi :]]).astype(np.float32)
        m_edge = np.ascontiguousarray(np.broadcast_to(me[None, :], (128, n_lo + n_hi)))

    in_maps = []
    for q in range(N_CORES):
        base = NC * q + d_lo
        chunk = sig_ext[base : base + 127 + F_S]
        s_hankel = np.ascontiguousarray(
            np.lib.stride_tricks.sliding_window_view(chunk, F_S)[:128]
        )
        im = {"s_hankel": s_hankel}
        if m_edge is not None:
            im["m_edge"] = m_edge
        in_maps.append(im)

    nc = _get_program(W, n_lo, n_hi)
    res = run_bass_kernel_spmd(nc, in_maps, list(range(N_CORES)), **RUN_KWARGS)
    global LAST_RESULTS
    LAST_RESULTS = res
    return np.concatenate(
        [
            res.results[q]["out_chunk"].astype(np.float32).T.reshape(NC)
            for q in range(N_CORES)
        ]
    )


# test-harness knobs (unused by graders): set RUN_KWARGS = {"trace": True}
# before calling kernel() to capture a profile in LAST_RESULTS.
RUN_KWARGS: dict = {}
LAST_RESULTS = None


# revision 11
# speedup vs baseline: 2.0973x; 1.6143x over previous
"""Trainium2 Bass kernel for nn_DifferentiableAlways (sparse_attention).

Math: the reference builds [2T,T] matrices, but column c of the output is just
    out[c] = -log( sum_{d in D} exp(-sig_ext[c+d] * m[d]) )
where m[d] = sigmoid(d - t_start) * sigmoid(t_end - d) (f32), D = {d: m[d] > 1e-3}
(a contiguous window), and sig_ext = concat(signal, full(T, signal[-1])).
Entries outside D are masked to 1e6 and contribute exp(-1e6) == 0 exactly in f32.

Inside D, m[d] == 1.0 exactly (saturated sigmoids) except for ~23 values at
each end of the window. So out[c] splits into
  core(c) = sum_{j=c+e_lo}^{c+e_hi} w(j),   w = exp(-sig_ext)   (m == 1 part)
  edge(c) = sum over ~46 edge d of exp(-sig_ext[c+d] * m[d])
and core(c) is a sliding-window sum: core(c) = P[c+e_hi] - P[c+e_lo-1] with P
the prefix sum of w. Each core computes a LOCAL prefix over only the range its
512 output columns need (so gather offsets are core-independent constants and
the program stays SPMD): exp on ScalarE -> free-dim running-sum scan on VectorE
-> cross-partition carry via one strict-lower-triangular matmul on PE ->
per-partition broadcast add -> DRAM roundtrip to regather the two shifted
windows as [NBLK, 128] tiles (contiguous 512B runs). The tail (sub/add/ln/neg)
runs in that transposed layout; the edge sums are moved across with one PE
matmul-transpose against an identity.

Raw Bass (explicit semaphores) because this container's walrus rejects
instructions carrying more than one semaphore wait, which Tile's auto-sync
emits.
"""

from contextlib import ExitStack

import numpy as np

import concourse.bass as bass
import concourse.mybir as mybir
from concourse.bass_utils import run_bass_kernel_spmd

T_DIM = 4096
N_CORES = 8
NC = T_DIM // N_CORES          # columns per core
NBLK = NC // 128               # 128-column blocks per core
LARGE_NUMBER = 1.0e6
DELTA = 1.0e-3
SCALE = 1.0

_F32 = mybir.dt.float32


def _build(W_core: int, n_lo: int, n_hi: int):
    """Per-core Bass program. W_core = saturated window length (m == 1.0),
    n_lo/n_hi = unsaturated edge columns at the window ends."""
    n_edge = n_lo + n_hi
    ne_all = n_edge * NBLK
    Exp = mybir.ActivationFunctionType.Exp
    Ln = mybir.ActivationFunctionType.Ln
    Copy = mybir.ActivationFunctionType.Copy

    # local prefix length: indices i in [0, 511 + W_core]
    R = -(-(NC + W_core) // 128) if W_core else 1

    nc = bass.Bass()
    sig_l = nc.dram_tensor("sig_local", [128, R], _F32, kind="ExternalInput")
    u_st = s_e = m_r = i_d = None
    if W_core:
        u_st = nc.dram_tensor("u_strict", [128, 128], _F32, kind="ExternalInput")
    if n_edge:
        s_e = nc.dram_tensor("s_edge", [128, ne_all], _F32, kind="ExternalInput")
        m_r = nc.dram_tensor("m_rep", [128, ne_all], _F32, kind="ExternalInput")
        i_d = nc.dram_tensor("ident", [128, 128], _F32, kind="ExternalInput")
    # out_chunk[b, p] = output for column 128*b + p of this core's slice
    out = nc.dram_tensor("out_chunk", [NBLK, 128], _F32, kind="ExternalOutput")
    p_dram = nc.dram_tensor("p_scratch", [128 * R], _F32)

    with ExitStack() as ctx:
        sig_sb = ctx.enter_context(nc.sbuf_tensor([128, R], _F32))
        w_sb = ctx.enter_context(nc.sbuf_tensor([128, R], _F32))
        scan_sb = ctx.enter_context(nc.sbuf_tensor([128, R], _F32))
        p2_sb = ctx.enter_context(nc.sbuf_tensor([128, R], _F32))
        u_sb = ctx.enter_context(nc.sbuf_tensor([128, 128], _F32))
        id_sb = ctx.enter_context(nc.sbuf_tensor([128, 128], _F32))
        excl_sb = ctx.enter_context(nc.sbuf_tensor([128, 1], _F32))
        hi_t = ctx.enter_context(nc.sbuf_tensor([NBLK, 128], _F32))
        lo_t = ctx.enter_context(nc.sbuf_tensor([NBLK, 128], _F32))
        se_sb = ctx.enter_context(nc.sbuf_tensor([128, max(ne_all, 1)], _F32))
        mr_sb = ctx.enter_context(nc.sbuf_tensor([128, max(ne_all, 1)], _F32))
        xe_sb = ctx.enter_context(nc.sbuf_tensor([128, max(ne_all, 1)], _F32))
        ee_sb = ctx.enter_context(nc.sbuf_tensor([128, max(ne_all, 1)], _F32))
        accE = ctx.enter_context(nc.sbuf_tensor([128, NBLK], _F32))
        core_t = ctx.enter_context(nc.sbuf_tensor([NBLK, 128], _F32))
        tot_t = ctx.enter_context(nc.sbuf_tensor([NBLK, 128], _F32))
        lg_t = ctx.enter_context(nc.sbuf_tensor([NBLK, 128], _F32))
        ng_t = ctx.enter_context(nc.sbuf_tensor([NBLK, 128], _F32))
        psum_excl = ctx.enter_context(nc.psum_tensor([128, 1], _F32))
        psum_aET = ctx.enter_context(nc.psum_tensor([NBLK, 128], _F32))

        sem_sig = ctx.enter_context(nc.semaphore("sem_sig"))
        sem_edge = ctx.enter_context(nc.semaphore("sem_edge"))
        pe_ready = ctx.enter_context(nc.semaphore("pe_ready"))
        pe2_ready = ctx.enter_context(nc.semaphore("pe2_ready"))
        exp_sem = ctx.enter_context(nc.semaphore("exp_sem"))
        mm_sem = ctx.enter_context(nc.semaphore("mm_sem"))
        mm2_sem = ctx.enter_context(nc.semaphore("mm2_sem"))
        p2_sem = ctx.enter_context(nc.semaphore("p2_sem"))
        dma_mid = ctx.enter_context(nc.semaphore("dma_mid"))
        dma_g = ctx.enter_context(nc.semaphore("dma_g"))
        mul_sem = ctx.enter_context(nc.semaphore("mul_sem"))
        tot_sem = ctx.enter_context(nc.semaphore("tot_sem"))
        fin_sem = ctx.enter_context(nc.semaphore("fin_sem"))
        dma_out = ctx.enter_context(nc.semaphore("dma_out"))
        block = ctx.enter_context(nc.Block(no_gpsimd_drain=True))

        @block.sync
        def _(sync):
            sync.dma_start(out=sig_sb[:], in_=sig_l[:]).then_inc(sem_sig, 16)
            if W_core:
                # PE waits pe_ready >= 17: 16 from this DMA + 1 from the scan
                sync.dma_start(out=u_sb[:], in_=u_st[:]).then_inc(pe_ready, 16)
            if n_edge:
                # PE waits pe2_ready >= 17: 16 here + 1 from the edge sums
                sync.dma_start(out=id_sb[:], in_=i_d[:]).then_inc(pe2_ready, 16)
                sync.dma_start(out=se_sb[:, 0:ne_all], in_=s_e[:]).then_inc(
                    sem_edge, 16
                )
                sync.dma_start(out=mr_sb[:, 0:ne_all], in_=m_r[:]).then_inc(
                    sem_edge, 16
                )
            if W_core:
                sync.wait_ge(p2_sem, 1)
                sync.dma_start(
                    out=p_dram[:].rearrange("(p r) -> p r", r=R), in_=p2_sb[:]
                ).then_inc(dma_mid, 16)
                sync.wait_ge(dma_mid, 16)
                sync.dma_start(
                    out=hi_t[:],
                    in_=p_dram[W_core : W_core + NC].rearrange("(b p) -> b p", p=128),
                ).then_inc(dma_g, 16)
                sync.wait_ge(dma_mid, 16)
                sync.dma_start(
                    out=lo_t[:],
                    in_=p_dram[0:NC].rearrange("(b p) -> b p", p=128),
                ).then_inc(dma_g, 16)
            sync.wait_ge(fin_sem, 1)
            sync.dma_start(out=out[:], in_=ng_t[:]).then_inc(dma_out, 16)
            sync.wait_ge(dma_out, 16)

        @block.vector
        def _(vector):
            if W_core:
                vector.wait_ge(exp_sem, 1)
                vector.tensor_tensor_scan(
                    scan_sb[:],
                    w_sb[:],
                    w_sb[:],
                    0.0,
                    mybir.AluOpType.add,
                    mybir.AluOpType.bypass,
                ).then_inc(pe_ready, 1)
                vector.wait_ge(mm_sem, 1)
                vector.tensor_copy(excl_sb[:], psum_excl[:])
                vector.tensor_scalar_add(p2_sb[:], scan_sb[:], excl_sb[:]).then_inc(
                    p2_sem, 1
                )
            if n_edge:
                vector.wait_ge(sem_edge, 32)
                vector.tensor_mul(
                    xe_sb[:, 0:ne_all], se_sb[:, 0:ne_all], mr_sb[:, 0:ne_all]
                ).then_inc(mul_sem, 1)
            if W_core:
                vector.wait_ge(dma_g, 32)
                vector.tensor_sub(core_t[:], hi_t[:], lo_t[:])
            else:
                vector.memset(core_t[:], 0.0)
            if n_edge:
                vector.wait_ge(mm2_sem, 1)
                vector.tensor_add(tot_t[:], core_t[:], psum_aET[:]).then_inc(
                    tot_sem, 1
                )
            else:
                vector.tensor_copy(tot_t[:], core_t[:]).then_inc(tot_sem, 1)

        @block.scalar
        def _(scalar):
            # Warm the exp/ln table load (~1.3us) under the input DMA:
            # scale=0.0 kills the data dependency.
            scalar.activation(lg_t[0:1, 0:1], lg_t[0:1, 0:1], Exp, scale=0.0)
            if W_core:
                scalar.wait_ge(sem_sig, 16)
                scalar.activation(w_sb[:], sig_sb[:], Exp, scale=-1.0).then_inc(
                    exp_sem, 1
                )
            if n_edge:
                scalar.wait_ge(mul_sem, 1)
                for b in range(NBLK):
                    col = n_edge * b
                    ins = scalar.activation(
                        ee_sb[:, col : col + n_edge],
                        xe_sb[:, col : col + n_edge],
                        Exp,
                        scale=-1.0,
                        accum_out=accE[:, b : b + 1],
                    )
                    if b == NBLK - 1:
                        ins.then_inc(pe2_ready, 1)
            scalar.wait_ge(tot_sem, 1)
            scalar.activation(lg_t[:], tot_t[:], Ln)
            scalar.activation(ng_t[:], lg_t[:], Copy, scale=-1.0).then_inc(fin_sem, 1)

        @block.tensor
        def _(tensor):
            if W_core:
                tensor.wait_ge(pe_ready, 17)
                tensor.matmul(
                    psum_excl[:], u_sb[:], scan_sb[:, R - 1 : R]
                ).then_inc(mm_sem, 1)
            if n_edge:
                # psum_aET[b, p] = accE[p, b]
                tensor.wait_ge(pe2_ready, 17)
                tensor.matmul(psum_aET[:], accE[:], id_sb[:]).then_inc(mm2_sem, 1)

    return nc


_cache: dict = {}


def _get_program(W_core, n_lo, n_hi):
    key = (W_core, n_lo, n_hi)
    if key not in _cache:
        _cache[key] = _build(W_core, n_lo, n_hi)
    return _cache[key]


def _sigmoid_f32(x64: np.ndarray) -> np.ndarray:
    return (1.0 / (1.0 + np.exp(-x64))).astype(np.float32)


def kernel(signal, t_start, t_end):
    signal = np.asarray(signal, dtype=np.float32).reshape(-1)
    T = signal.shape[0]
    assert T == T_DIM, f"expected T={T_DIM}, got {T}"
    ts = float(np.asarray(t_start).reshape(()))
    te = float(np.asarray(t_end).reshape(()))

    d64 = np.arange(T, dtype=np.float64)
    m = (_sigmoid_f32(SCALE * (d64 - ts)) * _sigmoid_f32(SCALE * (te - d64))).astype(
        np.float32
    )
    in_window = m > np.float32(DELTA)
    if not in_window.any():
        # every entry masked to LARGE_NUMBER: out = LARGE - log(2T)
        val = np.float32(LARGE_NUMBER) - np.float32(np.log(np.float32(2 * T)))
        return np.full(T, val, dtype=np.float32)

    idx = np.nonzero(in_window)[0]
    d_lo, d_hi = int(idx[0]), int(idx[-1])
    W = d_hi - d_lo + 1
    assert bool(in_window[d_lo : d_hi + 1].all()), "mask window not contiguous"

    m_win = m[d_lo : d_hi + 1]
    sat = m_win == np.float32(1.0)
    if sat.any():
        si = np.nonzero(sat)[0]
        n_lo, n_hi = int(si[0]), int(W - 1 - si[-1])
        assert bool(sat[si[0] : si[-1] + 1].all()), "saturated core not contiguous"
    else:
        n_lo, n_hi = W, 0  # everything goes through the explicit-multiply path
    n_edge = n_lo + n_hi
    W_core = W - n_edge
    e_lo = d_lo + n_lo  # first saturated d

    R = -(-(NC + W_core) // 128) if W_core else 1

    # sig_ext1[1 + j] = sig_ext[j]; the +1 absorbs the "-1" prefix-window start
    sig_ext1 = np.zeros(1 + T + NC * (N_CORES - 1) + d_hi + 128 * R + 256, np.float32)
    sig_ext1[1 : T + 1] = signal
    sig_ext1[T + 1 : 2 * T + 1] = signal[-1]
    # beyond 2T the values are never used by any in-range (c, d); zeros keep
    # exp() finite in the scanned-but-unread tail.

    d_edge = np.concatenate(
        [np.arange(d_lo, e_lo), np.arange(e_lo + W_core, d_hi + 1)]
    ).astype(np.int64)
    m_edge_vals = np.concatenate([m_win[:n_lo], m_win[W - n_hi :]]).astype(np.float32)
    m_rep = None
    if n_edge:
        m_rep = np.ascontiguousarray(
            np.broadcast_to(np.tile(m_edge_vals, NBLK)[None, :], (128, n_edge * NBLK))
        )

    u_strict = ident = None
    k = np.arange(128)
    if W_core:
        u_strict = (k[:, None] < k[None, :]).astype(np.float32)
    if n_edge:
        ident = (k[:, None] == k[None, :]).astype(np.float32)

    p_idx = np.arange(128)
    in_maps = []
    for q in range(N_CORES):
        cb = NC * q
        im = {}
        # local prefix source: w-index i maps to sig_ext1[cb + e_lo + i]
        start = cb + e_lo
        im["sig_local"] = sig_ext1[start : start + 128 * R].reshape(128, R).copy()
        if u_strict is not None:
            im["u_strict"] = u_strict
        if n_edge:
            im["ident"] = ident
            bb = np.arange(NBLK)
            idx3 = (
                1
                + cb
                + 128 * bb[None, :, None]
                + p_idx[:, None, None]
                + d_edge[None, None, :]
            )
            im["s_edge"] = np.ascontiguousarray(
                sig_ext1[idx3].reshape(128, NBLK * n_edge)
            )
            im["m_rep"] = m_rep
        in_maps.append(im)

    nc = _get_program(W_core, n_lo, n_hi)
    res = run_bass_kernel_spmd(nc, in_maps, list(range(N_CORES)), **RUN_KWARGS)
    global LAST_RESULTS
    LAST_RESULTS = res
    return np.concatenate(
        [res.results[q]["out_chunk"].astype(np.float32).reshape(NC) for q in range(N_CORES)]
    )


# test-harness knobs (unused by graders): set RUN_KWARGS = {"trace": True}
# before calling kernel() to capture a profile in LAST_RESULTS.
RUN_KWARGS: dict = {}
LAST_RESULTS = None
